# revision 10
# baseline (speedup 1.0000x reference)
"""Trainium2 Bass kernel for ExcitationEmbedding + Ion RoPE.

Computes, for inputs
  excitations [256, 512, 2] int64 (pairs (a, b) with a, b in [0, 6)),
  n_electrons [256] f32, n_protons [256] f32,
  emb_weight  [26, 256] f32, lookup_table [6, 6] int64:

  idx   = lookup_table[a, b]                       # [B, N]
  emb   = emb_weight[idx]                          # [B, N, D]
  out   = per-batch block-diagonal rotation of emb (theta from n_electrons,
          phi from n_protons, 4-wide blocks: dims (0,1) by theta, (2,3) by phi)

Strategy (pure data parallel over 8 cores, 32 batches each):
  - flat code f = 6*a + b in [0, 36); one-hot against an iota constant.
  - A 36-row phase-permuted table E_ph[j, p*64+k] = emb_weight[lut[j], 4k+p]
    is built once per core with a select-matmul (invalid j rows are 0).
  - Per batch, the rotated table rot36 is built with cheap [36, 64] ops:
    ACT scale-copies (sin terms) + DVE scalar_tensor_tensor (cos term + add).
  - Gather is a PE matmul: out_chunk[128tok, 256] = onehot_chunk.T @ rot36.
    PSUM evacuation copies un-permute the phase layout; one contiguous
    512 KB DMA per batch writes the output.
"""

import functools

import numpy as np

import concourse.bass as bass
import concourse.bacc as bacc
import concourse.mybir as mybir
from concourse import tile
from concourse.bass_utils import run_bass_kernel_spmd

B, N, D = 256, 512, 256
N_CORES = 8
BL = B // N_CORES  # 32 batches per core
ANGLE_SCALE = 0.05
HALF_PI = float(np.pi / 2)

F32 = mybir.dt.float32
I32 = mybir.dt.int32
AF = mybir.ActivationFunctionType
ALU = mybir.AluOpType


def build_bass() -> bass.Bass:
    nc = bacc.Bacc(
        "TRN2", target_bir_lowering=False, debug=False, num_devices=N_CORES
    )

    exc = nc.dram_tensor("exc", [BL, N * 2], I32, kind="ExternalInput")
    ne = nc.dram_tensor("ne", [1, BL], F32, kind="ExternalInput")
    npr = nc.dram_tensor("npr", [1, BL], F32, kind="ExternalInput")
    emb = nc.dram_tensor("emb", [26, D], F32, kind="ExternalInput")
    lut = nc.dram_tensor("lut", [1, 36], F32, kind="ExternalInput")
    out = nc.dram_tensor("out", [BL, N, D], F32, kind="ExternalOutput")

    iota36 = nc.inline_tensor(np.arange(36, dtype=np.float32).reshape(36, 1), "iota36")

    with tile.TileContext(nc) as tc:
        with (
            tc.tile_pool(name="const", bufs=1) as const,
            tc.tile_pool(name="work", bufs=3) as work,
            tc.tile_pool(name="opool", bufs=3) as opool,
            tc.tile_pool(name="dram", bufs=1, space="DRAM") as dram,
            tc.tile_pool(name="psum_s", bufs=1, space="PSUM") as psum_s,
            tc.tile_pool(name="psum", bufs=4, space="PSUM") as psum,
        ):
            # ---- loads ----
            exc_s = const.tile([BL, N * 2], I32)
            nc.sync.dma_start(out=exc_s[:], in_=exc[:])
            ne_s = const.tile([1, BL], F32)
            nc.sync.dma_start(out=ne_s[:], in_=ne[:])
            npr_s = const.tile([1, BL], F32)
            nc.sync.dma_start(out=npr_s[:], in_=npr[:])
            emb_s = const.tile([26, D], F32)
            nc.sync.dma_start(out=emb_s[:], in_=emb[:])
            lut_s = const.tile([1, 36], F32)
            nc.sync.dma_start(out=lut_s[:], in_=lut[:])
            iota_s = const.tile([36, 1], F32)
            nc.sync.dma_start(out=iota_s[:], in_=iota36[:])

            # ---- per-batch angles: rows of cos/sin values, [1, BL] each ----
            # layout in cs row: [ct | st | nst | cp | sp | nsp]
            hp = const.tile([1, 1], F32)
            nc.vector.memset(hp[:], HALF_PI)
            row6 = const.tile([1, 6 * BL], F32)
            # cos(t) = sin(pi/2 - t) keeps the LUT argument within [-pi, pi]
            nc.scalar.activation(row6[:, 0 * BL:1 * BL], ne_s[:], AF.Sin,
                                 bias=hp[:], scale=-ANGLE_SCALE)
            nc.scalar.activation(row6[:, 1 * BL:2 * BL], ne_s[:], AF.Sin,
                                 bias=0.0, scale=ANGLE_SCALE)
            nc.scalar.activation(row6[:, 2 * BL:3 * BL], ne_s[:], AF.Sin,
                                 bias=0.0, scale=-ANGLE_SCALE)
            nc.scalar.activation(row6[:, 3 * BL:4 * BL], npr_s[:], AF.Sin,
                                 bias=hp[:], scale=-ANGLE_SCALE)
            nc.scalar.activation(row6[:, 4 * BL:5 * BL], npr_s[:], AF.Sin,
                                 bias=0.0, scale=ANGLE_SCALE)
            nc.scalar.activation(row6[:, 5 * BL:6 * BL], npr_s[:], AF.Sin,
                                 bias=0.0, scale=-ANGLE_SCALE)
            row6_d = dram.tile([1, 6 * BL], F32)
            nc.sync.dma_start(out=row6_d[:], in_=row6[:])
            cs36 = const.tile([36, 6 * BL], F32)
            nc.sync.dma_start(out=cs36[:], in_=row6_d[:].to_broadcast((36, 6 * BL)))

            # ---- select matrix: selT[r, j] = (lut_flat[j] == r), r in [0,26) ----
            lut_bc = const.tile([26, 36], F32)
            nc.sync.dma_start(out=lut_bc[:], in_=lut[0:1, :].to_broadcast((26, 36)))
            selT = const.tile([26, 36], F32)
            nc.vector.tensor_scalar(out=selT[:], in0=lut_bc[:],
                                    scalar1=iota_s[0:26, :], scalar2=None,
                                    op0=ALU.is_equal)

            # ---- phase-permuted 36-row table: e_ph[j, p*64+k] = emb36[j, 4k+p]
            emb_perm = emb_s[:].rearrange("r (k p) -> r k p", p=4).transpose([0, 2, 1])
            eph_ps = psum_s.tile([36, D], F32)
            nc.tensor.matmul(eph_ps[:], selT[:], emb_perm, start=True, stop=True)
            e_ph = const.tile([36, D], F32)
            nc.scalar.activation(e_ph[:], eph_ps[:], AF.Copy)

            # ---- flat codes f32: flat[b, n] = 6*a + b ----
            exc3 = exc_s[:].rearrange("q (n two) -> q n two", two=2)
            a_f = const.tile([BL, N], F32)
            nc.vector.tensor_copy(a_f[:], exc3[:, :, 0])
            b_f = const.tile([BL, N], F32)
            nc.vector.tensor_copy(b_f[:], exc3[:, :, 1])
            flat = const.tile([BL, N], F32)
            nc.vector.scalar_tensor_tensor(out=flat[:], in0=a_f[:], scalar=6.0,
                                           in1=b_f[:], op0=ALU.mult,
                                           op1=ALU.add)
            flat_d = dram.tile([BL, N], F32)
            nc.sync.dma_start(out=flat_d[:], in_=flat[:])

            # phase ph: rot[:, ph*64:(ph+1)*64] = c * main + s * partner
            # (c, s) column offsets into cs36: ct=0, st=1, nst=2, cp=3, sp=4, nsp=5
            PH = [(0, 0, 1, 1), (1, 0, 0, 2), (2, 3, 3, 4), (3, 3, 2, 5)]

            for b in range(BL):
                flat_bc = work.tile([36, N], F32)
                nc.sync.dma_start(out=flat_bc[:],
                                  in_=flat_d[b:b + 1, :].to_broadcast((36, N)))
                onehot = work.tile([36, N], F32)
                nc.vector.tensor_scalar(out=onehot[:], in0=flat_bc[:],
                                        scalar1=iota_s[:], scalar2=None,
                                        op0=ALU.is_equal)

                rot = work.tile([36, D], F32)
                for (mph, coff, pph, soff) in PH:
                    mslice = e_ph[:, mph * 64:(mph + 1) * 64]
                    pslice = e_ph[:, pph * 64:(pph + 1) * 64]
                    c_ap = cs36[:, coff * BL + b:coff * BL + b + 1]
                    s_ap = cs36[:, soff * BL + b:soff * BL + b + 1]
                    tmp = work.tile([36, 64], F32, tag="tmp", bufs=4)
                    nc.scalar.activation(tmp[:], pslice, AF.Copy, scale=s_ap)
                    nc.vector.scalar_tensor_tensor(
                        out=rot[:, mph * 64:(mph + 1) * 64], in0=mslice,
                        scalar=c_ap, in1=tmp[:], op0=ALU.mult, op1=ALU.add)

                obuf = opool.tile([128, 4 * D], F32)
                for c in range(4):
                    ps = psum.tile([128, D], F32)
                    nc.tensor.matmul(ps[:], onehot[:, c * 128:(c + 1) * 128],
                                     rot[:], start=True, stop=True)
                    # un-permute phases: psum col p*64+k -> output dim 4k+p
                    dst = obuf[:, c * D:(c + 1) * D].rearrange(
                        "t (k p) -> t k p", p=4).transpose([0, 2, 1])
                    if c == 0:
                        nc.vector.tensor_copy(dst, ps[:])
                    else:
                        nc.scalar.activation(dst, ps[:], AF.Copy)

                nc.sync.dma_start(
                    out=out[b].rearrange("(c t) d -> t c d", t=128),
                    in_=obuf[:])

    nc.compile()
    return nc


@functools.lru_cache(maxsize=1)
def _get_nc() -> bass.Bass:
    return build_bass()


def kernel_with_results(excitations, n_electrons, n_protons, emb_weight,
                        lookup_table, trace=False):
    exc = np.ascontiguousarray(np.asarray(excitations)).astype(np.int64)
    exc32 = exc.astype(np.int32).reshape(B, N * 2)
    ne = np.ascontiguousarray(np.asarray(n_electrons, dtype=np.float32))
    npr = np.ascontiguousarray(np.asarray(n_protons, dtype=np.float32))
    emb = np.ascontiguousarray(np.asarray(emb_weight, dtype=np.float32))
    lut_f = np.asarray(lookup_table).astype(np.float32).reshape(1, 36)
    lut_f = np.ascontiguousarray(lut_f)

    in_maps = []
    for c in range(N_CORES):
        sl = slice(c * BL, (c + 1) * BL)
        in_maps.append({
            "exc": np.ascontiguousarray(exc32[sl]),
            "ne": np.ascontiguousarray(ne[sl].reshape(1, BL)),
            "npr": np.ascontiguousarray(npr[sl].reshape(1, BL)),
            "emb": emb,
            "lut": lut_f,
        })

    nc = _get_nc()
    res = run_bass_kernel_spmd(nc, in_maps, list(range(N_CORES)), trace=trace)
    out = np.concatenate([res.results[c]["out"] for c in range(N_CORES)], axis=0)
    return np.ascontiguousarray(out.reshape(B, N, D).astype(np.float32)), res


def kernel(excitations, n_electrons, n_protons, emb_weight, lookup_table):
    out, _ = kernel_with_results(excitations, n_electrons, n_protons,
                                 emb_weight, lookup_table)
    return out


# revision 14
# speedup vs baseline: 1.4002x; 1.4002x over previous
"""Trainium2 Bass kernel for ExcitationEmbedding + Ion RoPE.

Computes, for inputs
  excitations [256, 512, 2] int64 (pairs (a, b) with a, b in [0, 6)),
  n_electrons [256] f32, n_protons [256] f32,
  emb_weight  [26, 256] f32, lookup_table [6, 6] int64:

  idx   = lookup_table[a, b]                       # [B, N]
  emb   = emb_weight[idx]                          # [B, N, D]
  out   = per-batch block-diagonal rotation of emb (theta from n_electrons,
          phi from n_protons, 4-wide blocks: dims (0,1) by theta, (2,3) by phi)

Strategy (pure data parallel over 8 cores, 32 batches each):
  - flat code f = 6*a + b in [0, 36); one-hot against an iota constant
    (single fused is_equal over all batches, fp16).
  - A 36-row phase-permuted table E_ph[j, p*64+k] = emb_weight[lut[j], 4k+p]
    is built once per core with a select-matmul (invalid j rows are 0),
    then tiled across batches via DMA broadcast; rotated tables for ALL
    batches are built with 3 big [36, 32*256] fp16 DVE ops.
  - Gather is a PE fp16 matmul: chunk c of batch b covers tokens {4k+c},
    so each PSUM evacuation lands 4 consecutive tokens per partition and
    the per-batch 512 KB output DMA is fully contiguous.
"""

import functools

import numpy as np

import concourse.bass as bass
import concourse.bacc as bacc
import concourse.mybir as mybir
from concourse import tile
from concourse.bass_utils import run_bass_kernel_spmd

B, N, D = 256, 512, 256
N_CORES = 8
BL = B // N_CORES  # 32 batches per core
ANGLE_SCALE = 0.05
HALF_PI = float(np.pi / 2)

F32 = mybir.dt.float32
F16 = mybir.dt.float16
I32 = mybir.dt.int32
AF = mybir.ActivationFunctionType
ALU = mybir.AluOpType

# evacuation-engine split: chunk c of batch b goes to DVE if EVAC_DVE[b][c]
EVAC_DVE = [[c == 0 or (c == 1 and b < 20) for c in range(4)] for b in range(BL)]


def build_bass() -> bass.Bass:
    nc = bacc.Bacc(
        "TRN2", target_bir_lowering=False, debug=False, num_devices=N_CORES
    )

    exc = nc.dram_tensor("exc", [BL, N * 2], I32, kind="ExternalInput")
    ne = nc.dram_tensor("ne", [BL, 1], F32, kind="ExternalInput")
    npr = nc.dram_tensor("npr", [BL, 1], F32, kind="ExternalInput")
    emb = nc.dram_tensor("emb", [26, D], F32, kind="ExternalInput")
    lut = nc.dram_tensor("lut", [1, 36], F32, kind="ExternalInput")
    out = nc.dram_tensor("out", [BL, N, D], F32, kind="ExternalOutput")

    iota_f32 = nc.inline_tensor(
        np.arange(36, dtype=np.float32).reshape(36, 1), "iota_f32")
    iota_f16 = nc.inline_tensor(
        np.arange(36, dtype=np.float16).reshape(36, 1), "iota_f16")

    with tile.TileContext(nc) as tc:
        with (
            tc.tile_pool(name="const", bufs=1) as const,
            tc.tile_pool(name="opool", bufs=3) as opool,
            tc.tile_pool(name="dram", bufs=1, space="DRAM") as dram,
            tc.tile_pool(name="psum_s", bufs=1, space="PSUM") as psum_s,
            tc.tile_pool(name="psum", bufs=6, space="PSUM") as psum,
        ):
            # ---- loads ----
            exc_s = const.tile([BL, N * 2], I32)
            nc.sync.dma_start(out=exc_s[:], in_=exc[:])
            ne_s = const.tile([BL, 1], F32)
            nc.sync.dma_start(out=ne_s[:], in_=ne[:])
            npr_s = const.tile([BL, 1], F32)
            nc.sync.dma_start(out=npr_s[:], in_=npr[:])
            emb_s = const.tile([26, D], F16)
            nc.gpsimd.dma_start(out=emb_s[:], in_=emb[:])  # casts f32->f16
            iota_s = const.tile([36, 1], F32)
            nc.sync.dma_start(out=iota_s[:], in_=iota_f32[:])
            iota_h = const.tile([36, 1], F16)
            nc.sync.dma_start(out=iota_h[:], in_=iota_f16[:])

            # ---- per-batch angle columns [BL, 1] ----
            hp = const.tile([BL, 1], F32)
            nc.vector.memset(hp[:], HALF_PI)
            hpc = hp[:]
            # cos(t) = sin(pi/2 - t) keeps the LUT argument within [-pi, pi]
            ct = const.tile([BL, 1], F32)
            nc.scalar.activation(ct[:], ne_s[:], AF.Sin, bias=hpc, scale=-ANGLE_SCALE)
            st = const.tile([BL, 1], F32)
            nc.scalar.activation(st[:], ne_s[:], AF.Sin, bias=0.0, scale=ANGLE_SCALE)
            nst = const.tile([BL, 1], F32)
            nc.scalar.activation(nst[:], ne_s[:], AF.Sin, bias=0.0, scale=-ANGLE_SCALE)
            cp = const.tile([BL, 1], F32)
            nc.scalar.activation(cp[:], npr_s[:], AF.Sin, bias=hpc, scale=-ANGLE_SCALE)
            sp = const.tile([BL, 1], F32)
            nc.scalar.activation(sp[:], npr_s[:], AF.Sin, bias=0.0, scale=ANGLE_SCALE)
            nsp = const.tile([BL, 1], F32)
            nc.scalar.activation(nsp[:], npr_s[:], AF.Sin, bias=0.0, scale=-ANGLE_SCALE)

            # C_all[b, :] = (ct,ct,cp,cp)*64 in phase-major layout:
            # cols [0:64]=ct, [64:128]=ct, [128:192]=cp, [192:256]=cp
            ones = const.tile([BL, 64], F16)
            nc.vector.memset(ones[:], 1.0)
            c_all = const.tile([BL, D], F16)
            s_all = const.tile([BL, D], F16)
            for i, col in enumerate([ct, ct, cp, cp]):
                nc.vector.tensor_scalar(out=c_all[:, i * 64:(i + 1) * 64],
                                        in0=ones[:], scalar1=col[:], scalar2=None,
                                        op0=ALU.mult)
            for i, col in enumerate([st, nst, sp, nsp]):
                nc.vector.tensor_scalar(out=s_all[:, i * 64:(i + 1) * 64],
                                        in0=ones[:], scalar1=col[:], scalar2=None,
                                        op0=ALU.mult)

            # ---- select matrix: selT[r, j] = (lut_flat[j] == r), r in [0,26) ----
            lut_bc = const.tile([26, 36], F32)
            nc.sync.dma_start(out=lut_bc[:], in_=lut[0:1, :].to_broadcast((26, 36)))
            selT = const.tile([26, 36], F16)
            nc.vector.tensor_scalar(out=selT[:], in0=lut_bc[:],
                                    scalar1=iota_s[0:26, :], scalar2=None,
                                    op0=ALU.is_equal)

            # ---- phase-permuted 36-row table: e_ph[j, p*64+k] = emb36[j, 4k+p]
            emb_perm = emb_s[:].rearrange("r (k p) -> r k p", p=4).transpose([0, 2, 1])
            eph_ps = psum_s.tile([36, D], F32)
            nc.tensor.matmul(eph_ps[:], selT[:], emb_perm, start=True, stop=True)
            e_ph = const.tile([36, D], F16)
            nc.scalar.activation(e_ph[:], eph_ps[:], AF.Copy)
            # swapped-phase table [E1|E0|E3|E2]
            e_sw = const.tile([36, D], F16)
            for dst_i, src_i in [(0, 1), (1, 0), (2, 3), (3, 2)]:
                nc.vector.tensor_copy(e_sw[:, dst_i * 64:(dst_i + 1) * 64],
                                      e_ph[:, src_i * 64:(src_i + 1) * 64])

            # ---- flat codes: flat[b, n] = 6*a + b (fp16, values < 36) ----
            exc3 = exc_s[:].rearrange("q (n two) -> q n two", two=2)
            a_f = const.tile([BL, N], F32)
            nc.vector.tensor_copy(a_f[:], exc3[:, :, 0])
            b_f = const.tile([BL, N], F32)
            nc.vector.tensor_copy(b_f[:], exc3[:, :, 1])
            flat = const.tile([BL, N], F16)
            nc.vector.scalar_tensor_tensor(out=flat[:], in0=a_f[:], scalar=6.0,
                                           in1=b_f[:], op0=ALU.mult, op1=ALU.add)

            # ---- DRAM bounce + broadcast-read of per-batch tables ----
            flat_d = dram.tile([BL, N], F16)
            nc.sync.dma_start(out=flat_d[:], in_=flat[:])
            c_all_d = dram.tile([BL, D], F16)
            nc.sync.dma_start(out=c_all_d[:], in_=c_all[:])
            s_all_d = dram.tile([BL, D], F16)
            nc.sync.dma_start(out=s_all_d[:], in_=s_all[:])
            e_ph_d = dram.tile([36, D], F16)
            nc.sync.dma_start(out=e_ph_d[:], in_=e_ph[:])
            e_sw_d = dram.tile([36, D], F16)
            nc.sync.dma_start(out=e_sw_d[:], in_=e_sw[:])

            flat_big = const.tile([36, BL, N], F16)
            nc.sync.dma_start(
                out=flat_big[:],
                in_=flat_d[:].unsqueeze(0).broadcast_to((36, BL, N)))
            cb = const.tile([36, BL, D], F16)
            nc.sync.dma_start(
                out=cb[:], in_=c_all_d[:].unsqueeze(0).broadcast_to((36, BL, D)))
            sb = const.tile([36, BL, D], F16)
            nc.sync.dma_start(
                out=sb[:], in_=s_all_d[:].unsqueeze(0).broadcast_to((36, BL, D)))
            emb_t = const.tile([36, BL, D], F16)
            nc.sync.dma_start(
                out=emb_t[:], in_=e_ph_d[:].unsqueeze(1).broadcast_to((36, BL, D)))
            emb_sw = const.tile([36, BL, D], F16)
            nc.sync.dma_start(
                out=emb_sw[:], in_=e_sw_d[:].unsqueeze(1).broadcast_to((36, BL, D)))

            # ---- one-hot for all batches (single op) ----
            onehot = const.tile([36, BL, N], F16)
            nc.vector.tensor_scalar(out=onehot[:], in0=flat_big[:],
                                    scalar1=iota_s[:], scalar2=None,
                                    op0=ALU.is_equal)

            # ---- rotated tables for all batches: rot = emb_t*cb + emb_sw*sb
            t1 = const.tile([36, BL, D], F16)
            nc.vector.tensor_mul(t1[:], emb_t[:], cb[:])
            t2 = const.tile([36, BL, D], F16)
            nc.vector.tensor_mul(t2[:], emb_sw[:], sb[:])
            rot = const.tile([36, BL, D], F16)
            nc.vector.tensor_add(rot[:], t1[:], t2[:])

            # ---- gather matmuls + evacuation + output DMA ----
            for b in range(BL):
                obuf = opool.tile([128, 4 * D], F32)
                for c in range(4):
                    ps = psum.tile([128, D], F32)
                    # chunk c covers tokens {4k + c}: stride-4 weight columns
                    nc.tensor.matmul(ps[:], onehot[:, b, c::4], rot[:, b, :],
                                     start=True, stop=True)
                    # un-permute phases: psum col p*64+k -> obuf col c*256+4k+p
                    dst = obuf[:, c * D:(c + 1) * D].rearrange(
                        "t (k p) -> t k p", p=4).transpose([0, 2, 1])
                    if EVAC_DVE[b][c]:
                        nc.vector.tensor_copy(dst, ps[:])
                    else:
                        nc.scalar.activation(dst, ps[:], AF.Copy)

                # token t = 4k + c lives at obuf[k, c*256:(c+1)*256] -> the
                # DRAM view below is fully linear (contiguous 512 KB write)
                nc.sync.dma_start(
                    out=out[b].rearrange("(p c) d -> p c d", p=128),
                    in_=obuf[:])

    nc.compile()
    return nc


@functools.lru_cache(maxsize=1)
def _get_nc() -> bass.Bass:
    return build_bass()


def kernel_with_results(excitations, n_electrons, n_protons, emb_weight,
                        lookup_table, trace=False):
    exc = np.ascontiguousarray(np.asarray(excitations)).astype(np.int64)
    exc32 = exc.astype(np.int32).reshape(B, N * 2)
    ne = np.ascontiguousarray(np.asarray(n_electrons, dtype=np.float32))
    npr = np.ascontiguousarray(np.asarray(n_protons, dtype=np.float32))
    emb = np.ascontiguousarray(np.asarray(emb_weight, dtype=np.float32))
    lut_f = np.asarray(lookup_table).astype(np.float32).reshape(1, 36)
    lut_f = np.ascontiguousarray(lut_f)

    in_maps = []
    for c in range(N_CORES):
        sl = slice(c * BL, (c + 1) * BL)
        in_maps.append({
            "exc": np.ascontiguousarray(exc32[sl]),
            "ne": np.ascontiguousarray(ne[sl].reshape(BL, 1)),
            "npr": np.ascontiguousarray(npr[sl].reshape(BL, 1)),
            "emb": emb,
            "lut": lut_f,
        })

    nc = _get_nc()
    res = run_bass_kernel_spmd(nc, in_maps, list(range(N_CORES)), trace=trace)
    out_arr = np.concatenate(
        [res.results[c]["out"] for c in range(N_CORES)], axis=0)
    return np.ascontiguousarray(out_arr.reshape(B, N, D).astype(np.float32)), res


def kernel(excitations, n_electrons, n_protons, emb_weight, lookup_table):
    out_arr, _ = kernel_with_results(excitations, n_electrons, n_protons,
                                     emb_weight, lookup_table)
    return out_arr
